# revision 19
# baseline (speedup 1.0000x reference)
"""Bass/Tile TRN2 kernel for nn_Attention_48653389529729.

reference (jax):
    cat = concat([broadcast(hidden, (S,B,H)), encoder_output], axis=2)  # [S,B,2H]
    energy = tanh(einsum("sbi,hi->sbh", cat, W_attn) + b_attn)          # [S,B,H]
    scores = einsum("sbh,h->sb", energy, v)                             # [S,B]
    out = softmax(scores.T, axis=1)[:, None, :]                        # [B,1,S]

Decomposition: W_attn = [Wh | We] (columns 0:H apply to hidden, H:2H to enc).
    a[b,h]   = hidden[b] @ Wh.T + b_attn   (tiny; precomputed on host)
    E[h,s|b] = We @ enc[:,b,:].T           (the big matmul, fp16 in / fp32 acc)
    scores[b,s] = v . tanh(E + a[b])       (tanh on ACT, v-dot on PE)

Sharding: data-parallel on B across 8 cores (32 b per core); We/v replicated.
Host-side prep (layout only): We is shipped pre-transposed [i, h], zero-padded
i 500->512, cast to fp16; a+b_attn shipped as [125, 4, 32] f32 per core; v as
[125, 4] fp16; a 128x128 fp16 identity for the PE transpose-mode.

Device layout: energyT [h(part), s(free)] so the 500-dim contraction sits on
partitions.  enc arrives [s(part), i(free)] as f32 via SWDGE DMAs, is cast
f32->fp16 on DVE (i zero-padded 500->512), transposed to [i(part), s(free)]
by PE transpose-mode (fp16: ~56ns per 128x128 tile), copied PSUM->SBUF by
DVE.  The contraction runs as 4 chunks of K=128; the output h dim as 4
chunks of M=125; N=512 (one PSUM bank).  PSUM accumulates in fp32.
"""

import sys

sys.path.insert(0, "/opt/trn_rl_repo")

import numpy as np

import concourse.mybir as mybir
import concourse.tile as tile
from concourse import bacc
from concourse.bass_utils import run_bass_kernel_spmd

F32 = mybir.dt.float32
F16 = mybir.dt.float16
TANH = mybir.ActivationFunctionType.Tanh
EXP = mybir.ActivationFunctionType.Exp

S, B, H = 512, 256, 500
NCORES = 8
BL = B // NCORES  # 32 batches per core
PC = 125          # h (output) chunk size: 500 = 4 * 125
KC = 128          # i (contraction) chunk size, zero-padded 500 -> 512
NKC = 4           # number of chunks
ST = 4            # s-tiles of 128 (512 = 4 * 128)
HP = NKC * KC     # padded i size (512)

_CACHE = {}


def _build(enc_bufs=3, enc16_bufs=3, encT_bufs=4, psumT_bufs=2, psumE_bufs=4,
           psumS_bufs=2, tanh_bufs=4):
    nc = bacc.Bacc("TRN2", target_bir_lowering=False)

    enc_d = nc.dram_tensor("enc", [S, BL, H], F32, kind="ExternalInput")
    weT_d = nc.dram_tensor("weT", [HP, HP], F16, kind="ExternalInput")
    ab_d = nc.dram_tensor("ab", [KC, NKC, BL], F32, kind="ExternalInput")
    v_d = nc.dram_tensor("v16", [KC, NKC], F16, kind="ExternalInput")
    id_d = nc.dram_tensor("ident", [128, 128], F16, kind="ExternalInput")
    out_d = nc.dram_tensor("out", [BL, 1, S], F32, kind="ExternalOutput")

    with tile.TileContext(nc) as tc:
        with (
            tc.tile_pool(name="singles", bufs=1) as singles,
            tc.tile_pool(name="encp", bufs=enc_bufs) as encp,
        ):
            def load_enc(bi):
                enc_f32 = encp.tile([128, ST, H], F32, tag="enc32")
                nc.gpsimd.dma_start(
                    out=enc_f32,
                    in_=enc_d[:, bi, :].rearrange("(t p) i -> p t i", p=128),
                )
                return enc_f32

            enc_tiles = {0: load_enc(0)}
            # weT[p, k, h] = We.T[128k + p, h]  (i on partitions, fp16)
            weT = singles.tile([KC, NKC, HP], F16)
            nc.gpsimd.dma_start(
                out=weT, in_=weT_d[:, :].rearrange("(k p) h -> p k h", p=KC)
            )
            for bi in (1, 2):
                enc_tiles[bi] = load_enc(bi)

            ident = singles.tile([128, 128], F16)
            nc.sync.dma_start(out=ident, in_=id_d[:, :])
            ab = singles.tile([KC, NKC, BL], F32)
            nc.sync.dma_start(out=ab, in_=ab_d[:, :, :])
            v_sb = singles.tile([KC, NKC], F16)
            nc.sync.dma_start(out=v_sb, in_=v_d[:, :])
            # preload the Exp activation table before the tail needs it
            exp_warm = singles.tile([1, 1], F32)
            nc.vector.memset(exp_warm, 0.0)
            nc.scalar.activation(
                out=exp_warm, in_=exp_warm, func=EXP, scale=1.0
            )

            # ---- main loop over local batches ----
            with (
                tc.tile_pool(name="enc16p", bufs=enc16_bufs) as enc16p,
                tc.tile_pool(name="encTp", bufs=encT_bufs) as encTp,
                tc.tile_pool(name="tanhp", bufs=tanh_bufs) as tanhp,
                tc.tile_pool(name="stripp", bufs=4) as stripp,
                tc.tile_pool(name="sm", bufs=2) as sm,
                tc.tile_pool(name="ps_T", bufs=psumT_bufs, space="PSUM") as ps_T,
                tc.tile_pool(name="ps_E", bufs=psumE_bufs, space="PSUM") as ps_E,
                tc.tile_pool(name="ps_S", bufs=psumS_bufs, space="PSUM") as ps_S,
            ):
                GRP = 8  # softmax group size
                sc_group = None
                for bi in range(BL):
                    if bi in enc_tiles:
                        enc_f32 = enc_tiles.pop(bi)
                    else:
                        enc_f32 = load_enc(bi)
                    # f32 -> fp16 cast on DVE, zero-padding i to 512
                    enc_nat = enc16p.tile([128, ST, HP], F16, tag="enc16")
                    nc.vector.tensor_copy(enc_nat[:, :, :H], enc_f32)
                    nc.vector.memset(enc_nat[:, :, H:], 0.0)
                    # transposes: t=0,1 via xbar DMA (idle sync queue),
                    # t=2,3 via PE transpose-mode + DVE copy
                    eall = encTp.tile([KC, NKC, ST, 128], F16, tag="encT")
                    for t in range(2):
                        nc.sync.dma_start(
                            out=eall[:, :, t, :],
                            in_=enc_nat[:, t, :],
                            transpose=True,
                        )
                    for kk in range(NKC // 2):
                        psT = ps_T.tile([KC, 2, 2, 128], F16, tag="psT")
                        for k2 in range(2):
                            k = 2 * kk + k2
                            for t in (2, 3):
                                nc.tensor.transpose(
                                    psT[:, k2, t - 2, :],
                                    enc_nat[:, t, KC * k : KC * (k + 1)],
                                    ident,
                                )
                        nc.vector.tensor_copy(
                            eall[:, 2 * kk : 2 * kk + 2, 2:, :], psT
                        )
                    encT = [eall[:, k, :, :] for k in range(NKC)]
                    ths = []
                    for m in range(NKC):
                        psE = ps_E.tile([KC, S], F32, tag="psE")
                        for k in range(NKC):
                            nc.tensor.matmul(
                                psE,
                                weT[:, k, KC * m : KC * (m + 1)],
                                encT[k],
                                start=(k == 0),
                                stop=(k == NKC - 1),
                            )
                        th = tanhp.tile([KC, S], F16, tag="tanh")
                        nc.scalar.activation(
                            out=th,
                            in_=psE,
                            func=TANH,
                            bias=ab[:, m, bi : bi + 1],
                            scale=1.0,
                        )
                        ths.append(th)
                    psS = ps_S.tile([1, S], F32, tag="psS")
                    for m in range(NKC):
                        nc.tensor.matmul(
                            psS,
                            v_sb[:, m : m + 1],
                            ths[m],
                            start=(m == 0),
                            stop=(m == NKC - 1),
                        )
                    if bi % GRP == 0:
                        sc_group = sm.tile([GRP, S], F32, tag="scg")
                    strip = stripp.tile([1, S], F32, tag="strip")
                    nc.vector.tensor_copy(strip, psS)
                    nc.gpsimd.dma_start(
                        out=sc_group[bi % GRP : bi % GRP + 1, :], in_=strip
                    )

                    if bi % GRP == GRP - 1:
                        # softmax for this group of GRP batches
                        g = bi - GRP + 1
                        negmax = sm.tile([GRP, 1], F32, tag="negmax")
                        nc.vector.reduce_max(
                            negmax,
                            sc_group,
                            axis=mybir.AxisListType.X,
                            negate=True,
                        )
                        probs = sm.tile([GRP, S], F32, tag="probs")
                        sums = sm.tile([GRP, 1], F32, tag="sums")
                        nc.scalar.activation(
                            out=probs,
                            in_=sc_group,
                            func=EXP,
                            bias=negmax,
                            scale=1.0,
                            accum_out=sums,
                        )
                        rinv = sm.tile([GRP, 1], F32, tag="rinv")
                        nc.vector.reciprocal(rinv, sums)
                        nc.vector.tensor_scalar_mul(probs, probs, rinv)
                        nc.sync.dma_start(
                            out=out_d[g : bi + 1, :, :],
                            in_=probs.rearrange("b (one s) -> b one s", one=1),
                        )

    nc.compile()
    return nc


def _get_nc(**kw):
    key = tuple(sorted(kw.items()))
    if key not in _CACHE:
        _CACHE[key] = _build(**kw)
    return _CACHE[key]


def kernel(hidden, encoder_output, W_attn, b_attn, v, **run_kw):
    hidden = np.asarray(hidden, dtype=np.float32)
    encoder_output = np.asarray(encoder_output, dtype=np.float32)
    W_attn = np.asarray(W_attn, dtype=np.float32)
    b_attn = np.asarray(b_attn, dtype=np.float32)
    v = np.asarray(v, dtype=np.float32)

    # host-side layout prep (cheap, one-shot)
    weT = np.zeros((HP, HP), dtype=np.float16)
    weT[:H, :H] = W_attn[:, H:].T.astype(np.float16)         # [i, h], padded
    a_full = np.zeros((B, HP), dtype=np.float32)
    a_full[:, :H] = hidden[0] @ W_attn[:, :H].T + b_attn     # [B, H] f32
    v16 = np.zeros((KC, NKC), dtype=np.float16)
    v16.reshape(-1)[: H // 4 * 4] = 0  # layout below
    vpad = np.zeros(HP, dtype=np.float32)
    vpad[:H] = v
    v16 = np.ascontiguousarray(vpad.reshape(NKC, KC).T).astype(np.float16)
    ident = np.eye(128, dtype=np.float16)

    nc = _get_nc()
    in_maps = []
    for c in range(NCORES):
        sl = slice(c * BL, (c + 1) * BL)
        ab_core = np.ascontiguousarray(
            a_full[sl].T.reshape(NKC, KC, BL).transpose(1, 0, 2)
        ).astype(np.float32)                                 # [128, 4, 32]
        in_maps.append(
            {
                "enc": np.ascontiguousarray(encoder_output[:, sl, :]),
                "weT": weT,
                "ab": ab_core,
                "v16": v16,
                "ident": ident,
            }
        )
    res = run_bass_kernel_spmd(
        nc, in_maps, core_ids=list(range(NCORES)), **run_kw
    )
    out = np.concatenate([res.results[c]["out"] for c in range(NCORES)], axis=0)
    if run_kw:
        return out.astype(np.float32), res
    return out.astype(np.float32)


# revision 20
# speedup vs baseline: 1.7087x; 1.7087x over previous
"""Bass/Tile TRN2 kernel for nn_Attention_48653389529729.

reference (jax):
    cat = concat([broadcast(hidden, (S,B,H)), encoder_output], axis=2)  # [S,B,2H]
    energy = tanh(einsum("sbi,hi->sbh", cat, W_attn) + b_attn)          # [S,B,H]
    scores = einsum("sbh,h->sb", energy, v)                             # [S,B]
    out = softmax(scores.T, axis=1)[:, None, :]                        # [B,1,S]

Decomposition: W_attn = [Wh | We] (columns 0:H apply to hidden, H:2H to enc).
    a[b,h]   = hidden[b] @ Wh.T + b_attn   (tiny; precomputed on host)
    E[h,s|b] = We @ enc[:,b,:].T           (the big matmul, fp16 in / fp32 acc)
    scores[b,s] = v . tanh(E + a[b])       (tanh on ACT, v-dot on PE)

Sharding: data-parallel on B across 8 cores (32 b per core); We/v replicated.
Host-side prep (layout only): We is shipped pre-transposed [i, h], zero-padded
i 500->512, cast to fp16; a+b_attn shipped as [125, 4, 32] f32 per core; v as
[125, 4] fp16; a 128x128 fp16 identity for the PE transpose-mode.

Device layout: energyT [h(part), s(free)] so the 500-dim contraction sits on
partitions.  enc arrives [s(part), i(free)] as f32 via SWDGE DMAs, is cast
f32->fp16 on DVE (i zero-padded 500->512), transposed to [i(part), s(free)]
by PE transpose-mode (fp16: ~56ns per 128x128 tile), copied PSUM->SBUF by
DVE.  The contraction runs as 4 chunks of K=128; the output h dim as 4
chunks of M=125; N=512 (one PSUM bank).  PSUM accumulates in fp32.
"""

import sys

sys.path.insert(0, "/opt/trn_rl_repo")

import numpy as np

import concourse.mybir as mybir
import concourse.tile as tile
from concourse import bacc
from concourse.bass_utils import run_bass_kernel_spmd

F32 = mybir.dt.float32
F16 = mybir.dt.float16
TANH = mybir.ActivationFunctionType.Tanh
EXP = mybir.ActivationFunctionType.Exp

S, B, H = 512, 256, 500
NCORES = 8
BL = B // NCORES  # 32 batches per core
PC = 125          # h (output) chunk size: 500 = 4 * 125
KC = 128          # i (contraction) chunk size, zero-padded 500 -> 512
NKC = 4           # number of chunks
ST = 4            # s-tiles of 128 (512 = 4 * 128)
HP = NKC * KC     # padded i size (512)

_CACHE = {}


def _build(enc_bufs=3, enc16_bufs=3, encT_bufs=4, psumT_bufs=2, psumE_bufs=4,
           psumS_bufs=2, tanh_bufs=4):
    nc = bacc.Bacc("TRN2", target_bir_lowering=False)

    enc_d = nc.dram_tensor("enc", [S, BL, H], F32, kind="ExternalInput")
    weT_d = nc.dram_tensor("weT", [HP, HP], F16, kind="ExternalInput")
    ab_d = nc.dram_tensor("ab", [KC, NKC, BL], F32, kind="ExternalInput")
    v_d = nc.dram_tensor("v16", [KC, NKC], F16, kind="ExternalInput")
    id_d = nc.dram_tensor("ident", [128, 128], F16, kind="ExternalInput")
    out_d = nc.dram_tensor("out", [BL, 1, S], F32, kind="ExternalOutput")

    with tile.TileContext(nc) as tc:
        with (
            tc.tile_pool(name="singles", bufs=1) as singles,
            tc.tile_pool(name="encp", bufs=enc_bufs) as encp,
        ):
            def load_enc(bi):
                enc_f32 = encp.tile([128, ST, H], F32, tag="enc32")
                nc.gpsimd.dma_start(
                    out=enc_f32,
                    in_=enc_d[:, bi, :].rearrange("(t p) i -> p t i", p=128),
                )
                return enc_f32

            enc_tiles = {0: load_enc(0)}
            # weT[p, k, h] = We.T[128k + p, h]  (i on partitions, fp16)
            weT = singles.tile([KC, NKC, HP], F16)
            nc.gpsimd.dma_start(
                out=weT, in_=weT_d[:, :].rearrange("(k p) h -> p k h", p=KC)
            )
            for bi in (1, 2):
                enc_tiles[bi] = load_enc(bi)

            ident = singles.tile([128, 128], F16)
            nc.sync.dma_start(out=ident, in_=id_d[:, :])
            ab = singles.tile([KC, NKC, BL], F32)
            nc.sync.dma_start(out=ab, in_=ab_d[:, :, :])
            v_sb = singles.tile([KC, NKC], F16)
            nc.sync.dma_start(out=v_sb, in_=v_d[:, :])
            # preload the Exp activation table before the tail needs it
            exp_warm = singles.tile([1, 1], F32)
            nc.vector.memset(exp_warm, 0.0)
            nc.scalar.activation(
                out=exp_warm, in_=exp_warm, func=EXP, scale=1.0
            )

            # ---- main loop over local batches ----
            with (
                tc.tile_pool(name="enc16p", bufs=enc16_bufs) as enc16p,
                tc.tile_pool(name="encTp", bufs=encT_bufs) as encTp,
                tc.tile_pool(name="tanhp", bufs=tanh_bufs) as tanhp,
                tc.tile_pool(name="stripp", bufs=4) as stripp,
                tc.tile_pool(name="sm", bufs=2) as sm,
                tc.tile_pool(name="ps_T", bufs=psumT_bufs, space="PSUM") as ps_T,
                tc.tile_pool(name="ps_E", bufs=psumE_bufs, space="PSUM") as ps_E,
                tc.tile_pool(name="ps_S", bufs=psumS_bufs, space="PSUM") as ps_S,
            ):
                GRP = 8  # softmax group size
                sc_group = None
                for bi in range(BL):
                    if bi in enc_tiles:
                        enc_f32 = enc_tiles.pop(bi)
                    else:
                        enc_f32 = load_enc(bi)
                    # f32 -> fp16 cast on DVE, zero-padding i to 512
                    enc_nat = enc16p.tile([128, ST, HP], F16, tag="enc16")
                    nc.vector.tensor_copy(enc_nat[:, :, :H], enc_f32)
                    nc.vector.memset(enc_nat[:, :, H:], 0.0)
                    # PE transposes, two k-chunks per PSUM bank
                    encT = []
                    for kk in range(NKC // 2):
                        psT = ps_T.tile([KC, 2, S], F16, tag="psT")
                        for k2 in range(2):
                            k = 2 * kk + k2
                            for t in range(ST):
                                nc.tensor.transpose(
                                    psT[:, k2, 128 * t : 128 * (t + 1)],
                                    enc_nat[:, t, KC * k : KC * (k + 1)],
                                    ident,
                                )
                        e = encTp.tile([KC, 2, S], F16, tag="encT")
                        nc.vector.tensor_copy(e, psT)
                        encT += [e[:, 0, :], e[:, 1, :]]
                    ths = []
                    for m in range(NKC):
                        psE = ps_E.tile([KC, S], F32, tag="psE")
                        for k in range(NKC):
                            nc.tensor.matmul(
                                psE,
                                weT[:, k, KC * m : KC * (m + 1)],
                                encT[k],
                                start=(k == 0),
                                stop=(k == NKC - 1),
                            )
                        th = tanhp.tile([KC, S], F16, tag="tanh")
                        nc.scalar.activation(
                            out=th,
                            in_=psE,
                            func=TANH,
                            bias=ab[:, m, bi : bi + 1],
                            scale=1.0,
                        )
                        ths.append(th)
                    psS = ps_S.tile([1, S], F32, tag="psS")
                    for m in range(NKC):
                        nc.tensor.matmul(
                            psS,
                            v_sb[:, m : m + 1],
                            ths[m],
                            start=(m == 0),
                            stop=(m == NKC - 1),
                        )
                    if bi % GRP == 0:
                        sc_group = sm.tile([GRP, S], F32, tag="scg")
                    strip = stripp.tile([1, S], F32, tag="strip")
                    nc.vector.tensor_copy(strip, psS)
                    nc.gpsimd.dma_start(
                        out=sc_group[bi % GRP : bi % GRP + 1, :], in_=strip
                    )

                    if bi % GRP == GRP - 1:
                        # softmax for this group of GRP batches
                        g = bi - GRP + 1
                        negmax = sm.tile([GRP, 1], F32, tag="negmax")
                        nc.vector.reduce_max(
                            negmax,
                            sc_group,
                            axis=mybir.AxisListType.X,
                            negate=True,
                        )
                        probs = sm.tile([GRP, S], F32, tag="probs")
                        sums = sm.tile([GRP, 1], F32, tag="sums")
                        nc.scalar.activation(
                            out=probs,
                            in_=sc_group,
                            func=EXP,
                            bias=negmax,
                            scale=1.0,
                            accum_out=sums,
                        )
                        rinv = sm.tile([GRP, 1], F32, tag="rinv")
                        nc.vector.reciprocal(rinv, sums)
                        nc.vector.tensor_scalar_mul(probs, probs, rinv)
                        nc.sync.dma_start(
                            out=out_d[g : bi + 1, :, :],
                            in_=probs.rearrange("b (one s) -> b one s", one=1),
                        )

    nc.compile()
    return nc


def _get_nc(**kw):
    key = tuple(sorted(kw.items()))
    if key not in _CACHE:
        _CACHE[key] = _build(**kw)
    return _CACHE[key]


def kernel(hidden, encoder_output, W_attn, b_attn, v, **run_kw):
    hidden = np.asarray(hidden, dtype=np.float32)
    encoder_output = np.asarray(encoder_output, dtype=np.float32)
    W_attn = np.asarray(W_attn, dtype=np.float32)
    b_attn = np.asarray(b_attn, dtype=np.float32)
    v = np.asarray(v, dtype=np.float32)

    # host-side layout prep (cheap, one-shot)
    weT = np.zeros((HP, HP), dtype=np.float16)
    weT[:H, :H] = W_attn[:, H:].T.astype(np.float16)         # [i, h], padded
    a_full = np.zeros((B, HP), dtype=np.float32)
    a_full[:, :H] = hidden[0] @ W_attn[:, :H].T + b_attn     # [B, H] f32
    v16 = np.zeros((KC, NKC), dtype=np.float16)
    v16.reshape(-1)[: H // 4 * 4] = 0  # layout below
    vpad = np.zeros(HP, dtype=np.float32)
    vpad[:H] = v
    v16 = np.ascontiguousarray(vpad.reshape(NKC, KC).T).astype(np.float16)
    ident = np.eye(128, dtype=np.float16)

    nc = _get_nc()
    in_maps = []
    for c in range(NCORES):
        sl = slice(c * BL, (c + 1) * BL)
        ab_core = np.ascontiguousarray(
            a_full[sl].T.reshape(NKC, KC, BL).transpose(1, 0, 2)
        ).astype(np.float32)                                 # [128, 4, 32]
        in_maps.append(
            {
                "enc": np.ascontiguousarray(encoder_output[:, sl, :]),
                "weT": weT,
                "ab": ab_core,
                "v16": v16,
                "ident": ident,
            }
        )
    res = run_bass_kernel_spmd(
        nc, in_maps, core_ids=list(range(NCORES)), **run_kw
    )
    out = np.concatenate([res.results[c]["out"] for c in range(NCORES)], axis=0)
    if run_kw:
        return out.astype(np.float32), res
    return out.astype(np.float32)


# revision 21
# speedup vs baseline: 1.8156x; 1.0626x over previous
"""Bass/Tile TRN2 kernel for nn_Attention_48653389529729.

reference (jax):
    cat = concat([broadcast(hidden, (S,B,H)), encoder_output], axis=2)  # [S,B,2H]
    energy = tanh(einsum("sbi,hi->sbh", cat, W_attn) + b_attn)          # [S,B,H]
    scores = einsum("sbh,h->sb", energy, v)                             # [S,B]
    out = softmax(scores.T, axis=1)[:, None, :]                        # [B,1,S]

Decomposition: W_attn = [Wh | We] (columns 0:H apply to hidden, H:2H to enc).
    a[b,h]   = hidden[b] @ Wh.T + b_attn   (tiny; precomputed on host)
    E[h,s|b] = We @ enc[:,b,:].T           (the big matmul, fp16 in / fp32 acc)
    scores[b,s] = v . tanh(E + a[b])       (tanh on ACT, v-dot on PE)

Sharding: data-parallel on B across 8 cores (32 b per core); We/v replicated.
Host-side prep (layout only): We is shipped pre-transposed [i, h], zero-padded
i 500->512, cast to fp16; a+b_attn shipped as [125, 4, 32] f32 per core; v as
[125, 4] fp16; a 128x128 fp16 identity for the PE transpose-mode.

Device layout: energyT [h(part), s(free)] so the 500-dim contraction sits on
partitions.  enc arrives [s(part), i(free)] as f32 via SWDGE DMAs, is cast
f32->fp16 on DVE (i zero-padded 500->512), transposed to [i(part), s(free)]
by PE transpose-mode (fp16: ~56ns per 128x128 tile), copied PSUM->SBUF by
DVE.  The contraction runs as 4 chunks of K=128; the output h dim as 4
chunks of M=125; N=512 (one PSUM bank).  PSUM accumulates in fp32.
"""

import sys

sys.path.insert(0, "/opt/trn_rl_repo")

import numpy as np

import concourse.mybir as mybir
import concourse.tile as tile
from concourse import bacc
from concourse.bass_utils import run_bass_kernel_spmd

F32 = mybir.dt.float32
F16 = mybir.dt.float16
TANH = mybir.ActivationFunctionType.Tanh
EXP = mybir.ActivationFunctionType.Exp

S, B, H = 512, 256, 500
NCORES = 8
BL = B // NCORES  # 32 batches per core
PC = 125          # h (output) chunk size: 500 = 4 * 125
KC = 128          # i (contraction) chunk size, zero-padded 500 -> 512
NKC = 4           # number of chunks
ST = 4            # s-tiles of 128 (512 = 4 * 128)
HP = NKC * KC     # padded i size (512)

_CACHE = {}


def _build(enc_bufs=4, enc16_bufs=4, encT_bufs=6, psumT_bufs=2, psumE_bufs=4,
           psumS_bufs=2, tanh_bufs=6, grp=4):
    nc = bacc.Bacc("TRN2", target_bir_lowering=False)

    enc_d = nc.dram_tensor("enc", [S, BL, H], F32, kind="ExternalInput")
    weT_d = nc.dram_tensor("weT", [HP, HP], F16, kind="ExternalInput")
    ab_d = nc.dram_tensor("ab", [KC, NKC, BL], F32, kind="ExternalInput")
    v_d = nc.dram_tensor("v16", [KC, NKC], F16, kind="ExternalInput")
    id_d = nc.dram_tensor("ident", [128, 128], F16, kind="ExternalInput")
    out_d = nc.dram_tensor("out", [BL, 1, S], F32, kind="ExternalOutput")

    with tile.TileContext(nc) as tc:
        with (
            tc.tile_pool(name="singles", bufs=1) as singles,
            tc.tile_pool(name="encp", bufs=enc_bufs) as encp,
        ):
            def load_enc(bi):
                enc_f32 = encp.tile([128, ST, H], F32, tag="enc32")
                nc.gpsimd.dma_start(
                    out=enc_f32,
                    in_=enc_d[:, bi, :].rearrange("(t p) i -> p t i", p=128),
                )
                return enc_f32

            enc_tiles = {0: load_enc(0)}
            # weT[p, k, h] = We.T[128k + p, h]  (i on partitions, fp16)
            weT = singles.tile([KC, NKC, HP], F16)
            nc.gpsimd.dma_start(
                out=weT, in_=weT_d[:, :].rearrange("(k p) h -> p k h", p=KC)
            )
            for bi in (1, 2):
                enc_tiles[bi] = load_enc(bi)

            ident = singles.tile([128, 128], F16)
            nc.sync.dma_start(out=ident, in_=id_d[:, :])
            ab = singles.tile([KC, NKC, BL], F32)
            nc.sync.dma_start(out=ab, in_=ab_d[:, :, :])
            v_sb = singles.tile([KC, NKC], F16)
            nc.sync.dma_start(out=v_sb, in_=v_d[:, :])
            # preload the Exp activation table before the tail needs it
            exp_warm = singles.tile([1, 1], F32)
            nc.vector.memset(exp_warm, 0.0)
            nc.scalar.activation(
                out=exp_warm, in_=exp_warm, func=EXP, scale=1.0
            )

            # ---- main loop over local batches ----
            with (
                tc.tile_pool(name="enc16p", bufs=enc16_bufs) as enc16p,
                tc.tile_pool(name="encTp", bufs=encT_bufs) as encTp,
                tc.tile_pool(name="tanhp", bufs=tanh_bufs) as tanhp,
                tc.tile_pool(name="stripp", bufs=4) as stripp,
                tc.tile_pool(name="sm", bufs=2) as sm,
                tc.tile_pool(name="ps_T", bufs=psumT_bufs, space="PSUM") as ps_T,
                tc.tile_pool(name="ps_E", bufs=psumE_bufs, space="PSUM") as ps_E,
                tc.tile_pool(name="ps_S", bufs=psumS_bufs, space="PSUM") as ps_S,
            ):
                GRP = grp  # softmax group size
                sc_group = None
                for bi in range(BL):
                    if bi in enc_tiles:
                        enc_f32 = enc_tiles.pop(bi)
                    else:
                        enc_f32 = load_enc(bi)
                    # f32 -> fp16 cast on DVE, zero-padding i to 512
                    enc_nat = enc16p.tile([128, ST, HP], F16, tag="enc16")
                    nc.vector.tensor_copy(enc_nat[:, :, :H], enc_f32)
                    nc.vector.memset(enc_nat[:, :, H:], 0.0)
                    # PE transposes, two k-chunks per PSUM bank
                    encT = []
                    for kk in range(NKC // 2):
                        psT = ps_T.tile([KC, 2, S], F16, tag="psT")
                        for k2 in range(2):
                            k = 2 * kk + k2
                            for t in range(ST):
                                nc.tensor.transpose(
                                    psT[:, k2, 128 * t : 128 * (t + 1)],
                                    enc_nat[:, t, KC * k : KC * (k + 1)],
                                    ident,
                                )
                        e = encTp.tile([KC, 2, S], F16, tag="encT")
                        nc.vector.tensor_copy(e, psT)
                        encT += [e[:, 0, :], e[:, 1, :]]
                    ths = []
                    for m in range(NKC):
                        psE = ps_E.tile([KC, S], F32, tag="psE")
                        for k in range(NKC):
                            nc.tensor.matmul(
                                psE,
                                weT[:, k, KC * m : KC * (m + 1)],
                                encT[k],
                                start=(k == 0),
                                stop=(k == NKC - 1),
                            )
                        th = tanhp.tile([KC, S], F16, tag="tanh")
                        nc.scalar.activation(
                            out=th,
                            in_=psE,
                            func=TANH,
                            bias=ab[:, m, bi : bi + 1],
                            scale=1.0,
                        )
                        ths.append(th)
                    psS = ps_S.tile([1, S], F32, tag="psS")
                    for m in range(NKC):
                        nc.tensor.matmul(
                            psS,
                            v_sb[:, m : m + 1],
                            ths[m],
                            start=(m == 0),
                            stop=(m == NKC - 1),
                        )
                    if bi % GRP == 0:
                        sc_group = sm.tile([GRP, S], F32, tag="scg")
                    strip = stripp.tile([1, S], F32, tag="strip")
                    nc.vector.tensor_copy(strip, psS)
                    nc.gpsimd.dma_start(
                        out=sc_group[bi % GRP : bi % GRP + 1, :], in_=strip
                    )

                    if bi % GRP == GRP - 1:
                        # softmax for this group of GRP batches
                        g = bi - GRP + 1
                        negmax = sm.tile([GRP, 1], F32, tag="negmax")
                        nc.vector.reduce_max(
                            negmax,
                            sc_group,
                            axis=mybir.AxisListType.X,
                            negate=True,
                        )
                        probs = sm.tile([GRP, S], F32, tag="probs")
                        sums = sm.tile([GRP, 1], F32, tag="sums")
                        nc.scalar.activation(
                            out=probs,
                            in_=sc_group,
                            func=EXP,
                            bias=negmax,
                            scale=1.0,
                            accum_out=sums,
                        )
                        rinv = sm.tile([GRP, 1], F32, tag="rinv")
                        nc.vector.reciprocal(rinv, sums)
                        nc.vector.tensor_scalar_mul(probs, probs, rinv)
                        nc.sync.dma_start(
                            out=out_d[g : bi + 1, :, :],
                            in_=probs.rearrange("b (one s) -> b one s", one=1),
                        )

    nc.compile()
    return nc


def _get_nc(**kw):
    key = tuple(sorted(kw.items()))
    if key not in _CACHE:
        _CACHE[key] = _build(**kw)
    return _CACHE[key]


def kernel(hidden, encoder_output, W_attn, b_attn, v, **run_kw):
    hidden = np.asarray(hidden, dtype=np.float32)
    encoder_output = np.asarray(encoder_output, dtype=np.float32)
    W_attn = np.asarray(W_attn, dtype=np.float32)
    b_attn = np.asarray(b_attn, dtype=np.float32)
    v = np.asarray(v, dtype=np.float32)

    # host-side layout prep (cheap, one-shot)
    weT = np.zeros((HP, HP), dtype=np.float16)
    weT[:H, :H] = W_attn[:, H:].T.astype(np.float16)         # [i, h], padded
    a_full = np.zeros((B, HP), dtype=np.float32)
    a_full[:, :H] = hidden[0] @ W_attn[:, :H].T + b_attn     # [B, H] f32
    v16 = np.zeros((KC, NKC), dtype=np.float16)
    v16.reshape(-1)[: H // 4 * 4] = 0  # layout below
    vpad = np.zeros(HP, dtype=np.float32)
    vpad[:H] = v
    v16 = np.ascontiguousarray(vpad.reshape(NKC, KC).T).astype(np.float16)
    ident = np.eye(128, dtype=np.float16)

    nc = _get_nc()
    in_maps = []
    for c in range(NCORES):
        sl = slice(c * BL, (c + 1) * BL)
        ab_core = np.ascontiguousarray(
            a_full[sl].T.reshape(NKC, KC, BL).transpose(1, 0, 2)
        ).astype(np.float32)                                 # [128, 4, 32]
        in_maps.append(
            {
                "enc": np.ascontiguousarray(encoder_output[:, sl, :]),
                "weT": weT,
                "ab": ab_core,
                "v16": v16,
                "ident": ident,
            }
        )
    res = run_bass_kernel_spmd(
        nc, in_maps, core_ids=list(range(NCORES)), **run_kw
    )
    out = np.concatenate([res.results[c]["out"] for c in range(NCORES)], axis=0)
    if run_kw:
        return out.astype(np.float32), res
    return out.astype(np.float32)
